# revision 30
# baseline (speedup 1.0000x reference)
"""Trainium2 Bass kernel for local-correlation + masked top-256 (sparse_attention).

Contract: kernel(**inputs) takes FULL unsharded inputs (pre, curr, mask, mode)
and returns the full output tuple (f, b), each [4, 256, 128, 128] f32.

Sharding: pure data parallel over (batch, H-half) -> 8 cores.

Per core:
  - L2-normalize pre/curr over C in fp32 (chunked: sumsq via ones-matmul,
    reciprocal on DVE, sqrt on Act), write bf16 row-group tiles; chunks are
    emitted interleaved with the row loop so early rows start immediately.
  - Per output row h: 17 bf16 Gram matmuls cur^T @ pre -> PSUM; bf16 staging
    tile round-trips through DRAM with a +1-element partition shear so the
    diagonal band co[w, dy*17+dx] comes back as one DMA ([128, 289] bf16).
  - y = co + 4*m (mask lifts m=1 values into [3,5], m=0 stay in [-1,1]).
    TRUNCATED descending sort: only ranks 0..255 are ever used by the
    reconstruction, so 32 max8/match_replace rounds in 4 phases of 8, with a
    tombstone compaction between phases (sign on Act, +1-scan on DVE, index
    arithmetic on Pool, strided i16 casts on Act, u16-pair local_scatter on
    Pool).  Widths shrink 289 -> 225 -> 161 -> 97.
  - Reconstruct both outputs from the shared truncated sort (q = 289-kappa):
      f[r] = max(cs4[r] - 6*[r>=kappa], min(scatter(cs4, j->j+q | j<kappa), 0))
      b[r] = max(scatter(cs, j->j-kappa | kappa<=j<256) - 10*[r>=q or
                 r>=256-kappa], min(cs[r], 0))
    with cs = sorted y (256 ranks), cs4 = cs - 4; scatters are gpsimd
    local_scatter with host-built per-partition int16 indices; the maxes run
    as a + relu(b - a) on Pool+Act so the DVE does nothing but sort.
  - PE transpose [w,256] -> [256,w] halves into one [128,512] tile, one
    combined f|b DMA out per row.
"""

import numpy as np

K = 8
KW = 17
D = KW * KW            # 289
TOPK = 256
B, C, H, W = 4, 256, 128, 128
N_CORES = 8
HSLICE = H // 2        # 64 rows per core
WP = W + 2 * K         # 144
HP = HSLICE + 2 * K    # 80
NROUND = 32            # truncated sort: ranks 0..255 only
CS_COLS = NROUND * 8   # 256
STG = KW * WP          # 2448
SCR_FLAT = 128 * (STG + 2)   # write view uses row pitch STG+1, read STG+2
M4W = D + 1            # m4 padded to even length (col 289 = -9 pad)

PHR = (8, 8, 8, 8)                       # rounds per phase
LIVE = (289, 225, 161, 97)               # live width at the start of each phase
CIN = (290, 225, 161)                    # compaction input width (c1 incl pad col)
ZCC = (226, 162, 98)                     # zc tile cols (live + 1 pad, even)

AX_M4 = 0
AX_HM6 = M4W                 # 290
AX_HM10 = AX_HM6 + TOPK      # 546
AX_IDXF = AX_HM10 + TOPK     # 802
AX_IDXB = AX_IDXF + TOPK     # 1058
AUXT = AX_IDXB + TOPK        # 1314 int16 elements per pixel

PRE_RPT = 3            # padded-pre rows per tile (3*144 f32 fits a PSUM bank)
PRE_NT = (HP + PRE_RPT - 1) // PRE_RPT   # 27 tiles of [128, <=432]
PRE_TCOLS = PRE_RPT * WP        # 432
PRE_LAST_CW = HP * WP - (PRE_NT - 1) * PRE_TCOLS  # 288 (last tile partial)
CUR_RPT = 4            # cur rows per tile
CUR_NT = HSLICE // CUR_RPT      # 16 tiles of [128, 512]
CUR_TCOLS = CUR_RPT * W         # 512

_CACHED = {"nc": None}


def _build_nc():
    import concourse.bacc as bacc
    import concourse.tile as tile
    import concourse.mybir as mybir

    f32 = mybir.dt.float32
    bf16 = mybir.dt.bfloat16
    i16 = mybir.dt.int16
    u16 = mybir.dt.uint16
    AF = mybir.ActivationFunctionType
    ALU = mybir.AluOpType

    nc = bacc.Bacc("TRN2", target_bir_lowering=False, debug=False,
                   enable_asserts=False, num_devices=N_CORES)

    pre_d = nc.dram_tensor("pre_pad", [C, HP, WP], f32, kind="ExternalInput").ap()
    cur_d = nc.dram_tensor("curr", [C, HSLICE, W], f32, kind="ExternalInput").ap()
    auxc_d = nc.dram_tensor("auxc", [HSLICE, W, AUXT], i16, kind="ExternalInput").ap()
    fb_d = nc.dram_tensor("fb_out", [2, TOPK, HSLICE, W], f32,
                          kind="ExternalOutput").ap()
    scr = [nc.dram_tensor(f"scr{i}", [SCR_FLAT], bf16, kind="Internal").ap()
           for i in range(6)]

    ident_d = nc.inline_tensor(np.eye(128, dtype=np.float32), name="ident")
    ones_col_d = nc.inline_tensor(np.ones((128, 1), np.float32), name="ones_col")
    ones_row_d = nc.inline_tensor(np.ones((1, 128), np.float32), name="ones_row")

    from contextlib import ExitStack
    with tile.TileContext(nc) as tc, ExitStack() as _stk:
        _pools = {}
        for _nm, _bufs, _spc in (
                ("pp", 1, None), ("ldp", 3, None), ("sqp", 2, None),
                ("invp", 1, None), ("stp", 2, None), ("ssq", 3, None), ("ssp", 2, "PSUM"),
                ("bcp", 1, "PSUM"), ("stgp", 2, None), ("cop", 6, None),
                ("yp", 6, None), ("csp", 8, None), ("cwp", 4, None),
                ("i2p", 3, None), ("zc1p", 3, None), ("zc2p", 4, None),
                ("zc3p", 4, None), ("rcp", 2, None),
                ("fbp", 2, None), ("trp", 2, None), ("gp", 3, "PSUM"),
                ("tp", 2, "PSUM")):
            kw = {"space": _spc} if _spc else {}
            _pools[_nm] = _stk.enter_context(
                tc.tile_pool(name=_nm, bufs=_bufs, **kw))
        pp, ldp, sqp, invp, stp, ssq, ssp, bcp, stgp, cop, yp, csp, cwp, \
            i2p, zc1p, zc2p, zc3p, rcp, fbp, trp, gp, tp = (
                _pools[n] for n in ("pp", "ldp", "sqp", "invp", "stp", "ssq",
                                    "ssp", "bcp", "stgp", "cop", "yp", "csp",
                                    "cwp", "i2p", "zc1p", "zc2p", "zc3p",
                                    "rcp", "fbp", "trp", "gp", "tp"))
        zcpools = (zc1p, zc2p, zc3p)
        if True:
            pre_nb = [[pp.tile([128, PRE_TCOLS], bf16, tag=f"pre{c}_{i}",
                               name=f"pre{c}_{i}") for i in range(PRE_NT)]
                      for c in range(2)]
            cur_nb = [[pp.tile([128, CUR_TCOLS], bf16, tag=f"cur{c}_{i}",
                               name=f"cur{c}_{i}") for i in range(CUR_NT)]
                      for c in range(2)]
            ident = pp.tile([128, 128], bf16, tag="ident", name="identt")
            identf = pp.tile([128, 128], f32, tag="identf", name="identf")
            ones_col = pp.tile([128, 1], f32, tag="onc", name="onc")
            ones_row = pp.tile([1, 128], f32, tag="onr", name="onr")
            ones290 = pp.tile([128, M4W], f32, tag="on290", name="on290")
            bias85 = pp.tile([128, 1], f32, tag="b85", name="b85")
            nc.gpsimd.memset(ones290[:, :], 1.0)
            nc.gpsimd.memset(bias85[:, :], 8.5)
            nc.sync.dma_start(identf[:, :], ident_d.ap())
            nc.scalar.activation(ident[:, :], identf[:, :], AF.Copy)
            nc.sync.dma_start(ones_col[:, :], ones_col_d.ap())
            nc.sync.dma_start(ones_row[:, :], ones_row_d.ap())

            pre_flat = pre_d.rearrange("c hh ww -> c (hh ww)")
            cur_flat = cur_d.rearrange("c hh ww -> c (hh ww)")

            def norm_p1(flat_d, i, cw0, cw):
                """phase 1: load fp32 chunk, sumsq over C (squares on Pool,
                reduce on PE), 1/sumsq on DVE straight from PSUM."""
                cs_ = slice(i * cw0, i * cw0 + cw)
                chunks = []
                for c in range(2):
                    ch = ldp.tile([128, 512], f32, tag=f"ch{c}", name=f"ch{c}")
                    nc.sync.dma_start(ch[:, 0:cw],
                                      flat_d[c * 128:(c + 1) * 128, cs_])
                    chunks.append(ch)
                nsub = (cw + 511) // 512
                sw = cw // nsub
                rcs = ssq.tile([1, 512], f32, tag="rcs", name="rcs")
                sss = []
                for s in range(nsub):
                    sub = slice(s * sw, (s + 1) * sw)
                    ss = ssp.tile([1, 512], f32, tag="ss", name="ss")
                    for c in range(2):
                        sq = sqp.tile([128, 512], f32, tag="sq", name="sq")
                        nc.gpsimd.tensor_tensor(out=sq[:, 0:sw],
                                                in0=chunks[c][:, sub],
                                                in1=chunks[c][:, sub],
                                                op=ALU.mult)
                        nc.tensor.matmul(ss[:, 0:sw], ones_col[:, :],
                                         sq[:, 0:sw],
                                         start=(c == 0), stop=(c == 1))
                    sss.append(ss)
                return chunks, rcs, nsub, sw, sss

            def norm_recips(state):
                chunks, rcs, nsub, sw, sss = state
                for s in range(nsub):
                    nc.vector.reciprocal(rcs[:, s * sw:(s + 1) * sw],
                                         sss[s][:, 0:sw])

            def norm_p2(dst_tiles, i, state):
                """phase 2 (two blocks later): sqrt, broadcast, multiply."""
                chunks, rcs, nsub, sw, _ = state
                for s in range(nsub):
                    sub = slice(s * sw, (s + 1) * sw)
                    srt = stp.tile([1, 512], f32, tag="srt", name="srt")
                    nc.scalar.activation(srt[:, 0:sw], rcs[:, sub], AF.Sqrt)
                    bc = bcp.tile([128, 512], f32, tag="bc", name="bc")
                    nc.tensor.matmul(bc[:, 0:sw], ones_row[:, :], srt[:, 0:sw],
                                     start=True, stop=True)
                    inv = invp.tile([128, 512], f32, tag="inv", name="inv")
                    nc.scalar.activation(inv[:, 0:sw], bc[:, 0:sw], AF.Copy)
                    for c in range(2):
                        nc.gpsimd.tensor_tensor(
                            out=dst_tiles[c][i][:, sub],
                            in0=chunks[c][:, sub],
                            in1=inv[:, 0:sw], op=ALU.mult)

            _p2q = []

            def do_chunk_p1(w, tblk):
                flat = pre_flat if w[0] == "pre" else cur_flat
                cw0 = PRE_TCOLS if w[0] == "pre" else CUR_TCOLS
                cw = cw0
                if w[0] == "pre" and w[1] == PRE_NT - 1:
                    cw = PRE_LAST_CW
                _p2q.append((w, norm_p1(flat, w[1], cw0, cw), tblk))

            def flush_p2(tnow):
                while _p2q and _p2q[0][2] <= tnow - 2:
                    w, state, _ = _p2q.pop(0)
                    tiles = pre_nb if w[0] == "pre" else cur_nb
                    norm_p2(tiles, w[1], state)

            def do_chunk(w):
                do_chunk_p1(w, -10)
                norm_recips(_p2q[-1][1])
                flush_p2(0)

            # prologue: everything preworks at blocks 0/1 touch (pre tiles
            # 0..7), with immediate norm_p2 so program order is write->read
            for w in ([("cur", 0)] + [("pre", i) for i in range(6)]
                      + [("cur", 1), ("pre", 6), ("pre", 7)]):
                do_chunk(w)
            # remaining stage-A chunks; a chunk's norm_p2 is emitted 2 blocks
            # after its p1, so the slack must be 3 tiles (9 rows) for the
            # 3-row pre tiles to be written before the gram that reads them
            due = {}
            hp, hc = 8, 2
            npairs_ = HSLICE // 2
            for t in range(npairs_):
                items = []
                need_cur = min((2 * t + 5) // CUR_RPT + 2, CUR_NT)
                need_pre = min((2 * t + 21) // PRE_RPT + 3, PRE_NT)
                while hc < need_cur:
                    items.append(("cur", hc)); hc += 1
                while hp < need_pre:
                    items.append(("pre", hp)); hp += 1
                due[t] = items

            _recq = []

            def emit_a(t):
                for w in due.get(t, []):
                    do_chunk_p1(w, t)
                    _recq.append(_p2q[-1][1])
                flush_p2(t)

            def emit_recips():
                while _recq:
                    norm_recips(_recq.pop(0))

            def prework(h):
                """gram + shear + aux DMA + y build; returns row state"""
                stage = stgp.tile([128, STG], bf16, tag="stage", name="stage")
                ct = cur_nb[0][h // CUR_RPT], cur_nb[1][h // CUR_RPT]
                co_ = slice((h % CUR_RPT) * W, (h % CUR_RPT + 1) * W)
                # batch gram rows sharing a pre tile into one matmul
                r = h
                while r <= h + KW - 1:
                    pt_ = r // PRE_RPT
                    r_end = min(pt_ * PRE_RPT + PRE_RPT - 1, h + KW - 1)
                    n = r_end - r + 1
                    po = (r % PRE_RPT) * WP
                    g = gp.tile([128, PRE_RPT * WP], f32, tag="g", name="g")
                    for c in range(2):
                        nc.tensor.matmul(
                            g[:, 0:n * WP], ct[c][:, co_],
                            pre_nb[c][pt_][:, po:po + n * WP],
                            start=(c == 0), stop=(c == 1))
                    dy0 = r - h
                    nc.scalar.activation(
                        stage[:, dy0 * WP:(dy0 + n) * WP],
                        g[:, 0:n * WP], AF.Copy)
                    r = r_end + 1
                sc = scr[h % 6]
                wview = sc[0:128 * (STG + 1)].rearrange("(p r) -> p r", r=STG + 1)
                nc.sync.dma_start(wview[:, 0:STG], stage[:, :])
                co = cop.tile([128, M4W], bf16, tag="co", name="co", bufs=2)
                rview = sc[:].rearrange("(p r) -> p r", r=STG + 2)
                rview = rview[:, 0:STG].rearrange("p (a b) -> p a b", b=WP)
                nc.sync.dma_start(co[:, 0:D], rview[:, :, 0:KW])
                axc = cop.tile([128, AUXT], i16, tag="axc", name="axc", bufs=10)
                nc.sync.dma_start(axc[:, :], auxc_d[h, :, :])
                y = yp.tile([128, M4W], f32, tag="y", name="y")
                nc.gpsimd.tensor_tensor(
                    out=y[:, 0:D], in0=co[:, 0:D],
                    in1=axc[:, AX_M4:AX_HM6].bitcast(bf16)[:, 0:D],
                    op=ALU.add)
                nc.gpsimd.memset(y[:, D:M4W], -9.0)
                return [y, None, axc]

            def phase_rounds(arr, lw, cs, r0, nr):
                for r in range(r0, r0 + nr):
                    nc.vector.max(cs[:, r * 8:(r + 1) * 8], arr[:, 0:lw])
                    if r != NROUND - 1:
                        nc.vector.match_replace(arr[:, 0:lw],
                                                cs[:, r * 8:(r + 1) * 8],
                                                arr[:, 0:lw], imm_value=-9.0)

            def compact_scan(arr, ci):
                """sign (Act) + counting scan (DVE): returns (sg unused, e-pre)"""
                w_in = CIN[ci]
                sg = cwp.tile([128, M4W], f32, tag="sg", name="sg")
                nc.scalar.activation(sg[:, 0:w_in], arr[:, 0:w_in],
                                     AF.Sign, bias=bias85[:, :])
                s = cwp.tile([128, M4W], f32, tag="s", name="s")
                nc.vector.tensor_tensor_scan(out=s[:, 0:w_in],
                                             data0=sg[:, 0:w_in],
                                             data1=ones290[:, 0:w_in],
                                             initial=0.0,
                                             op0=ALU.add, op1=ALU.add)
                return sg, s

            def compact_fin(arr, ci, sg, s):
                """index arithmetic (Pool), strided i16 casts (Act), u16-pair
                scatter (Pool).  Returns the compacted zc tile."""
                w_in = CIN[ci]
                e = cwp.tile([128, M4W], f32, tag="e", name="e")
                nc.gpsimd.tensor_scalar(out=e[:, 0:w_in], in0=sg[:, 0:w_in],
                                        scalar1=10000.0, scalar2=None,
                                        op0=ALU.mult)
                nc.gpsimd.tensor_tensor(out=e[:, 0:w_in], in0=e[:, 0:w_in],
                                        in1=s[:, 0:w_in], op=ALU.add)
                i2 = i2p.tile([128, 2 * M4W], i16, tag="i2", name="i2")
                i2v = i2[:, 0:2 * w_in].rearrange("p (n two) -> p n two",
                                                  two=2)
                nc.scalar.activation(i2v[:, :, 0], e[:, 0:w_in], AF.Copy,
                                     bias=-10002.0)
                nc.scalar.activation(i2v[:, :, 1], e[:, 0:w_in], AF.Copy,
                                     bias=-10001.0)
                zc = zcpools[ci].tile([128, ZCC[ci]], f32, tag=f"zc{ci}",
                                      name=f"zc{ci}")
                nc.gpsimd.local_scatter(
                    zc[:, :].bitcast(u16), arr[:, 0:w_in].bitcast(u16),
                    i2[:, 0:2 * w_in], channels=128,
                    num_elems=2 * ZCC[ci], num_idxs=2 * w_in)
                return zc

            def pmax(out, a, bb, d, r):
                """out = max(a, b) without DVE: d = b - a (Pool),
                r = relu(d) (Act), out = a + r (Pool)."""
                nc.gpsimd.tensor_tensor(out=d[:, :], in0=bb[:, :], in1=a[:, :],
                                        op=ALU.subtract)
                nc.scalar.activation(r[:, :], d[:, :], AF.Relu)
                nc.gpsimd.tensor_tensor(out=out[:, :], in0=a[:, :],
                                        in1=r[:, :], op=ALU.add)

            def postwork(h, state):
                """casts, scatters, combines, transpose, combined output DMA"""
                _, cs, axc = state
                cs4b = rcp.tile([128, TOPK], bf16, tag="cs4b", name="cs4b")
                nc.scalar.activation(cs4b[:, :], cs[:, 0:TOPK], AF.Copy,
                                     bias=-4.0)
                csb = rcp.tile([128, TOPK], bf16, tag="csb", name="csb")
                nc.scalar.activation(csb[:, :], cs[:, 0:TOPK], AF.Copy)
                tf = rcp.tile([128, TOPK], bf16, tag="tf", name="tf")
                nc.gpsimd.local_scatter(tf[:, :], cs4b[:, :],
                                        axc[:, AX_IDXF:AX_IDXB], channels=128,
                                        num_elems=TOPK, num_idxs=TOPK)
                yb = rcp.tile([128, TOPK], bf16, tag="yb", name="yb")
                nc.gpsimd.local_scatter(yb[:, :], csb[:, :],
                                        axc[:, AX_IDXB:AUXT], channels=128,
                                        num_elems=TOPK, num_idxs=TOPK)
                axb = axc[:, 0:AX_IDXF].bitcast(bf16)
                # f = max(cs4b - hm6, min(tf, 0))
                fmain = fbp.tile([128, TOPK], bf16, tag="fm", name="fm")
                nc.gpsimd.tensor_tensor(out=fmain[:, :], in0=cs4b[:, :],
                                        in1=axb[:, AX_HM6:AX_HM10],
                                        op=ALU.subtract)
                nc.gpsimd.tensor_scalar_min(tf[:, :], tf[:, :], 0.0)
                ft = fbp.tile([128, TOPK], bf16, tag="ft", name="ft")
                dd = fbp.tile([128, TOPK], f32, tag="dd", name="dd")
                rr = fbp.tile([128, TOPK], f32, tag="rr", name="rr")
                pmax(ft, fmain, tf, dd, rr)
                # b = max(yb - hm10, min(csb, 0))
                bmain = fbp.tile([128, TOPK], bf16, tag="bm", name="bm")
                nc.gpsimd.tensor_tensor(out=bmain[:, :], in0=yb[:, :],
                                        in1=axb[:, AX_HM10:AX_IDXF],
                                        op=ALU.subtract)
                minb = fbp.tile([128, TOPK], bf16, tag="mb", name="mb")
                nc.gpsimd.tensor_scalar_min(minb[:, :], csb[:, :], 0.0)
                bt = fbp.tile([128, TOPK], bf16, tag="bt", name="bt")
                dd2 = fbp.tile([128, TOPK], f32, tag="dd2", name="dd2")
                rr2 = fbp.tile([128, TOPK], f32, tag="rr2", name="rr2")
                pmax(bt, bmain, minb, dd2, rr2)
                tr = trp.tile([128, 2 * TOPK], f32, tag="tr", name="tr")
                for oi, x in ((0, ft), (1, bt)):
                    for half in range(2):
                        pt = tp.tile([128, 128], bf16, tag="pt", name="pt")
                        nc.tensor.transpose(
                            pt[:, :], x[:, half * 128:(half + 1) * 128],
                            ident[:, :])
                        base = oi * TOPK + half * 128
                        nc.scalar.activation(tr[:, base:base + 128], pt[:, :],
                                             AF.Copy)
                oview = fb_d.rearrange("two (cc p) hh ww -> p two cc hh ww",
                                       cc=2)
                nc.sync.dma_start(
                    oview[:, :, :, h, :],
                    tr[:, :].rearrange("p (two cc ww) -> p two cc ww",
                                       two=2, ww=W))

            npairs = HSLICE // 2

            # two-pair software pipeline: pair t runs phases P0+P1 in block t
            # and P2+P3 in block t+1, so every compaction chain (Act sign ->
            # DVE scan -> Pool index math -> Act casts -> Pool scatter) hides
            # behind ~8-9us of the other pair's DVE rounds.  prework for pair
            # t+2 is emitted mid-block so its Act stage copies queue after the
            # signs the DVE scans are waiting on.
            pend = {0: (prework(0), prework(1)), 1: (prework(2), prework(3))}
            mid = {}
            old2 = {}
            for t in range(npairs + 2):
                cur = pend.pop(t, None)          # runs P0, c1, P1, c2
                mid_p = mid.pop(t - 1, None)     # runs P2, c3
                old_p = old2.pop(t - 2, None)    # runs P3, postwork
                # cur P0 (rows A then B), then the two c1 scans
                if cur is not None:
                    for st in cur:
                        st[1] = csp.tile([128, CS_COLS], f32, tag="cs",
                                         name="cs")
                        phase_rounds(st[0], LIVE[0], st[1], 0, PHR[0])
                    scans0 = [compact_scan(st[0], 0) for st in cur]
                # mid P2 + c3 scans
                if mid_p is not None:
                    for ri, st in enumerate(mid_p["st"]):
                        phase_rounds(mid_p["arrs"][ri], LIVE[2], st[1], 16,
                                     PHR[2])
                    scans2 = [compact_scan(mid_p["arrs"][ri], 2)
                              for ri in range(2)]
                # old P3 (zc3 was compacted last block -> no chain wait)
                if old_p is not None:
                    for ri, st in enumerate(old_p["st"]):
                        phase_rounds(old_p["arrs"][ri], LIVE[3], st[1], 24,
                                     PHR[3])
                # cur c1 fins -> zc1 (chain hides behind P2 + P3 above)
                if cur is not None:
                    arrs1 = [compact_fin(cur[ri][0], 0, *scans0[ri])
                             for ri in range(2)]
                # norm chunks + prework for pair t+2: AFTER the signs/casts so
                # the Act queue serves the sort's compaction chain first (the
                # stage copies have ~3 blocks of slack)
                if t + 2 <= npairs - 1:
                    emit_a(t)
                    pend[t + 2] = (prework(2 * t + 4), prework(2 * t + 5))
                emit_recips()
                # cur P1 + c2 scans
                if cur is not None:
                    for ri, st in enumerate(cur):
                        phase_rounds(arrs1[ri], LIVE[1], st[1], 8, PHR[1])
                    scans1 = [compact_scan(arrs1[ri], 1) for ri in range(2)]
                # mid c3 fins -> zc3 (consumed as P3 next block)
                if mid_p is not None:
                    arrs3 = [compact_fin(mid_p["arrs"][ri], 2, *scans2[ri])
                             for ri in range(2)]
                    old2[t] = {"st": mid_p["st"], "arrs": arrs3}
                # cur c2 fins -> zc2 (consumed as P2 next block)
                if cur is not None:
                    arrs2 = [compact_fin(arrs1[ri], 1, *scans1[ri])
                             for ri in range(2)]
                    mid[t] = {"st": cur, "arrs": arrs2}
                # old postwork
                if old_p is not None:
                    postwork(2 * (t - 2), old_p["st"][0])
                    postwork(2 * (t - 2) + 1, old_p["st"][1])
            for t in range(npairs - 2, npairs_):
                emit_a(t)
            flush_p2(10**6)
    nc.compile()
    return nc


def _host_prep(pre, curr, mask):
    import ml_dtypes
    bf = ml_dtypes.bfloat16

    pre_pad = np.pad(pre, ((0, 0), (0, 0), (K, K), (K, K)), mode="reflect")
    mask_pad = np.pad(mask, ((0, 0), (0, 0), (K, K), (K, K)))
    ins = []
    jf = np.arange(TOPK, dtype=np.int32)[None, :]
    for k in range(N_CORES):
        b, hh = k // 2, k % 2
        h0 = hh * HSLICE
        mp = mask_pad[b, 0, h0:h0 + HP, :]
        s0, s1 = mp.strides
        m_unf = np.lib.stride_tricks.as_strided(
            mp, (HSLICE, KW, W, KW), (s0, s0, s1, s1))
        m_unf = np.ascontiguousarray(
            m_unf.transpose(0, 2, 1, 3).reshape(HSLICE, W, D))
        kap = m_unf.sum(axis=2).astype(np.int32).reshape(-1, 1)   # [HS*W, 1]
        q = D - kap
        idxf = np.where((jf < kap) & (jf + q < TOPK), jf + q, -1)
        idxb = np.where(jf >= kap, jf - kap, -1)
        auxi = np.concatenate([idxf, idxb], axis=1).astype(np.int16)
        hm6 = np.where(jf >= kap, np.float32(6.0), np.float32(0.0))
        hm10 = np.where((jf >= q) | (jf >= TOPK - kap),
                        np.float32(10.0), np.float32(0.0))
        m4 = np.full((HSLICE * W, M4W), -9.0, np.float32)
        m4[:, :D] = 4.0 * m_unf.reshape(-1, D)
        # y = co + m4: pad col 289 gets co_pad(=0 in bf16 tile) + (-9) = -9...
        # co tile col 289 is uninitialized; the kernel memsets y[:,289] anyway.
        m4[:, D] = 0.0
        auxb = np.concatenate([m4, hm6, hm10], axis=1).astype(bf)
        auxc = np.concatenate([auxb.view(np.int16), auxi], axis=1)
        ins.append({
            "pre_pad": np.ascontiguousarray(pre_pad[b, :, h0:h0 + HP, :]),
            "curr": np.ascontiguousarray(curr[b, :, h0:h0 + HSLICE, :]),
            "auxc": np.ascontiguousarray(auxc.reshape(HSLICE, W, AUXT)),
        })
    return ins


def kernel(pre, curr, mask, mode):
    from concourse.bass_utils import run_bass_kernel_spmd

    pre = np.asarray(pre, dtype=np.float32)
    curr = np.asarray(curr, dtype=np.float32)
    mask = np.asarray(mask, dtype=np.float32)
    assert int(np.asarray(mode)) == 0

    if _CACHED["nc"] is None:
        _CACHED["nc"] = _build_nc()
    nc = _CACHED["nc"]

    in_maps = _host_prep(pre, curr, mask)
    res = run_bass_kernel_spmd(nc, in_maps, core_ids=list(range(N_CORES)))
    f = np.zeros((B, TOPK, H, W), np.float32)
    bo = np.zeros((B, TOPK, H, W), np.float32)
    for k in range(N_CORES):
        bb, hh = k // 2, k % 2
        fb = res.results[k]["fb_out"]
        f[bb, :, hh * HSLICE:(hh + 1) * HSLICE, :] = fb[0]
        bo[bb, :, hh * HSLICE:(hh + 1) * HSLICE, :] = fb[1]
    return (f, bo)


# revision 31
# speedup vs baseline: 1.0249x; 1.0249x over previous
"""Trainium2 Bass kernel for local-correlation + masked top-256 (sparse_attention).

Contract: kernel(**inputs) takes FULL unsharded inputs (pre, curr, mask, mode)
and returns the full output tuple (f, b), each [4, 256, 128, 128] f32.

Sharding: pure data parallel over (batch, H-half) -> 8 cores.

Per core:
  - L2-normalize pre/curr over C in fp32 (chunked: sumsq via ones-matmul,
    reciprocal on DVE, sqrt on Act), write bf16 row-group tiles; chunks are
    emitted interleaved with the row loop so early rows start immediately.
  - Per output row h: 17 bf16 Gram matmuls cur^T @ pre -> PSUM; bf16 staging
    tile round-trips through DRAM with a +1-element partition shear so the
    diagonal band co[w, dy*17+dx] comes back as one DMA ([128, 289] bf16).
  - y = co + 4*m (mask lifts m=1 values into [3,5], m=0 stay in [-1,1]).
    TRUNCATED descending sort: only ranks 0..255 are ever used by the
    reconstruction, so 32 max8/match_replace rounds in 4 phases of 8, with a
    tombstone compaction between phases (sign on Act, +1-scan on DVE, index
    arithmetic on Pool, strided i16 casts on Act, u16-pair local_scatter on
    Pool).  Widths shrink 289 -> 225 -> 161 -> 97.
  - Reconstruct both outputs from the shared truncated sort (q = 289-kappa):
      f[r] = max(cs4[r] - 6*[r>=kappa], min(scatter(cs4, j->j+q | j<kappa), 0))
      b[r] = max(scatter(cs, j->j-kappa | kappa<=j<256) - 10*[r>=q or
                 r>=256-kappa], min(cs[r], 0))
    with cs = sorted y (256 ranks), cs4 = cs - 4; scatters are gpsimd
    local_scatter with host-built per-partition int16 indices; the maxes run
    as a + relu(b - a) on Pool+Act so the DVE does nothing but sort.
  - PE transpose [w,256] -> [256,w] halves into one [128,512] tile, one
    combined f|b DMA out per row.
"""

import numpy as np

K = 8
KW = 17
D = KW * KW            # 289
TOPK = 256
B, C, H, W = 4, 256, 128, 128
N_CORES = 8
HSLICE = H // 2        # 64 rows per core
WP = W + 2 * K         # 144
HP = HSLICE + 2 * K    # 80
NROUND = 32            # truncated sort: ranks 0..255 only
CS_COLS = NROUND * 8   # 256
STG = KW * WP          # 2448
SCR_FLAT = 128 * (STG + 2)   # write view uses row pitch STG+1, read STG+2
M4W = D + 1            # m4 padded to even length (col 289 = -9 pad)

PHR = (8, 8, 8, 8)                       # rounds per phase
LIVE = (289, 225, 161, 97)               # live width at the start of each phase
CIN = (290, 225, 161)                    # compaction input width (c1 incl pad col)
ZCC = (226, 162, 98)                     # zc tile cols (live + 1 pad, even)

AX_M4 = 0
AX_HM6 = M4W                 # 290
AX_HM10 = AX_HM6 + TOPK      # 546
AX_IDXF = AX_HM10 + TOPK     # 802
AX_IDXB = AX_IDXF + TOPK     # 1058
AUXT = AX_IDXB + TOPK        # 1314 int16 elements per pixel

PRE_RPT = 3            # padded-pre rows per tile (3*144 f32 fits a PSUM bank)
PRE_NT = (HP + PRE_RPT - 1) // PRE_RPT   # 27 tiles of [128, <=432]
PRE_TCOLS = PRE_RPT * WP        # 432
PRE_LAST_CW = HP * WP - (PRE_NT - 1) * PRE_TCOLS  # 288 (last tile partial)
CUR_RPT = 4            # cur rows per tile
CUR_NT = HSLICE // CUR_RPT      # 16 tiles of [128, 512]
CUR_TCOLS = CUR_RPT * W         # 512

_CACHED = {"nc": None}


def _build_nc():
    import concourse.bacc as bacc
    import concourse.tile as tile
    import concourse.mybir as mybir

    f32 = mybir.dt.float32
    bf16 = mybir.dt.bfloat16
    i16 = mybir.dt.int16
    u16 = mybir.dt.uint16
    AF = mybir.ActivationFunctionType
    ALU = mybir.AluOpType

    nc = bacc.Bacc("TRN2", target_bir_lowering=False, debug=False,
                   enable_asserts=False, num_devices=N_CORES)

    pre_d = nc.dram_tensor("pre_pad", [C, HP, WP], f32, kind="ExternalInput").ap()
    cur_d = nc.dram_tensor("curr", [C, HSLICE, W], f32, kind="ExternalInput").ap()
    auxc_d = nc.dram_tensor("auxc", [HSLICE, W, AUXT], i16, kind="ExternalInput").ap()
    fb_d = nc.dram_tensor("fb_out", [2, TOPK, HSLICE, W], f32,
                          kind="ExternalOutput").ap()
    scr = [nc.dram_tensor(f"scr{i}", [SCR_FLAT], bf16, kind="Internal").ap()
           for i in range(6)]

    ident_d = nc.inline_tensor(np.eye(128, dtype=np.float32), name="ident")
    ones_col_d = nc.inline_tensor(np.ones((128, 1), np.float32), name="ones_col")
    ones_row_d = nc.inline_tensor(np.ones((1, 128), np.float32), name="ones_row")

    from contextlib import ExitStack
    with tile.TileContext(nc) as tc, ExitStack() as _stk:
        _pools = {}
        for _nm, _bufs, _spc in (
                ("pp", 1, None), ("ldp", 3, None), ("sqp", 2, None),
                ("invp", 1, None), ("stp", 2, None), ("ssq", 3, None), ("ssp", 2, "PSUM"),
                ("bcp", 1, "PSUM"), ("stgp", 2, None), ("cop", 6, None),
                ("yp", 6, None), ("csp", 6, None), ("cwp", 5, None),
                ("i2p", 5, None), ("zc1p", 3, None), ("zc2p", 4, None),
                ("zc3p", 3, None), ("rcp", 3, None),
                ("fbp", 2, None), ("trp", 2, None), ("gp", 3, "PSUM"),
                ("tp", 2, "PSUM")):
            kw = {"space": _spc} if _spc else {}
            _pools[_nm] = _stk.enter_context(
                tc.tile_pool(name=_nm, bufs=_bufs, **kw))
        pp, ldp, sqp, invp, stp, ssq, ssp, bcp, stgp, cop, yp, csp, cwp, \
            i2p, zc1p, zc2p, zc3p, rcp, fbp, trp, gp, tp = (
                _pools[n] for n in ("pp", "ldp", "sqp", "invp", "stp", "ssq",
                                    "ssp", "bcp", "stgp", "cop", "yp", "csp",
                                    "cwp", "i2p", "zc1p", "zc2p", "zc3p",
                                    "rcp", "fbp", "trp", "gp", "tp"))
        zcpools = (zc1p, zc2p, zc3p)
        if True:
            pre_nb = [[pp.tile([128, PRE_TCOLS], bf16, tag=f"pre{c}_{i}",
                               name=f"pre{c}_{i}") for i in range(PRE_NT)]
                      for c in range(2)]
            cur_nb = [[pp.tile([128, CUR_TCOLS], bf16, tag=f"cur{c}_{i}",
                               name=f"cur{c}_{i}") for i in range(CUR_NT)]
                      for c in range(2)]
            ident = pp.tile([128, 128], bf16, tag="ident", name="identt")
            identf = pp.tile([128, 128], f32, tag="identf", name="identf")
            ones_col = pp.tile([128, 1], f32, tag="onc", name="onc")
            ones_row = pp.tile([1, 128], f32, tag="onr", name="onr")
            ones290 = pp.tile([128, M4W], f32, tag="on290", name="on290")
            bias85 = pp.tile([128, 1], f32, tag="b85", name="b85")
            nc.gpsimd.memset(ones290[:, :], 1.0)
            nc.gpsimd.memset(bias85[:, :], 8.5)
            nc.sync.dma_start(identf[:, :], ident_d.ap())
            nc.scalar.activation(ident[:, :], identf[:, :], AF.Copy)
            nc.sync.dma_start(ones_col[:, :], ones_col_d.ap())
            nc.sync.dma_start(ones_row[:, :], ones_row_d.ap())

            pre_flat = pre_d.rearrange("c hh ww -> c (hh ww)")
            cur_flat = cur_d.rearrange("c hh ww -> c (hh ww)")

            def norm_p1(flat_d, i, cw0, cw):
                """phase 1: load fp32 chunk, sumsq over C (squares on Pool,
                reduce on PE), 1/sumsq on DVE straight from PSUM."""
                cs_ = slice(i * cw0, i * cw0 + cw)
                chunks = []
                for c in range(2):
                    ch = ldp.tile([128, 512], f32, tag=f"ch{c}", name=f"ch{c}")
                    nc.sync.dma_start(ch[:, 0:cw],
                                      flat_d[c * 128:(c + 1) * 128, cs_])
                    chunks.append(ch)
                nsub = (cw + 511) // 512
                sw = cw // nsub
                rcs = ssq.tile([1, 512], f32, tag="rcs", name="rcs")
                sss = []
                for s in range(nsub):
                    sub = slice(s * sw, (s + 1) * sw)
                    ss = ssp.tile([1, 512], f32, tag="ss", name="ss")
                    for c in range(2):
                        sq = sqp.tile([128, 512], f32, tag="sq", name="sq")
                        nc.gpsimd.tensor_tensor(out=sq[:, 0:sw],
                                                in0=chunks[c][:, sub],
                                                in1=chunks[c][:, sub],
                                                op=ALU.mult)
                        nc.tensor.matmul(ss[:, 0:sw], ones_col[:, :],
                                         sq[:, 0:sw],
                                         start=(c == 0), stop=(c == 1))
                    sss.append(ss)
                return chunks, rcs, nsub, sw, sss

            def norm_recips(state):
                chunks, rcs, nsub, sw, sss = state
                for s in range(nsub):
                    nc.vector.reciprocal(rcs[:, s * sw:(s + 1) * sw],
                                         sss[s][:, 0:sw])

            def norm_p2(dst_tiles, i, state):
                """phase 2 (two blocks later): sqrt, broadcast, multiply."""
                chunks, rcs, nsub, sw, _ = state
                for s in range(nsub):
                    sub = slice(s * sw, (s + 1) * sw)
                    srt = stp.tile([1, 512], f32, tag="srt", name="srt")
                    nc.scalar.activation(srt[:, 0:sw], rcs[:, sub], AF.Sqrt)
                    bc = bcp.tile([128, 512], f32, tag="bc", name="bc")
                    nc.tensor.matmul(bc[:, 0:sw], ones_row[:, :], srt[:, 0:sw],
                                     start=True, stop=True)
                    inv = invp.tile([128, 512], f32, tag="inv", name="inv")
                    nc.scalar.activation(inv[:, 0:sw], bc[:, 0:sw], AF.Copy)
                    for c in range(2):
                        nc.gpsimd.tensor_tensor(
                            out=dst_tiles[c][i][:, sub],
                            in0=chunks[c][:, sub],
                            in1=inv[:, 0:sw], op=ALU.mult)

            _p2q = []

            def do_chunk_p1(w, tblk):
                flat = pre_flat if w[0] == "pre" else cur_flat
                cw0 = PRE_TCOLS if w[0] == "pre" else CUR_TCOLS
                cw = cw0
                if w[0] == "pre" and w[1] == PRE_NT - 1:
                    cw = PRE_LAST_CW
                _p2q.append((w, norm_p1(flat, w[1], cw0, cw), tblk))

            def flush_p2(tnow):
                while _p2q and _p2q[0][2] <= tnow - 2:
                    w, state, _ = _p2q.pop(0)
                    tiles = pre_nb if w[0] == "pre" else cur_nb
                    norm_p2(tiles, w[1], state)

            def do_chunk(w):
                do_chunk_p1(w, -10)
                norm_recips(_p2q[-1][1])
                flush_p2(0)

            # prologue: everything preworks at blocks 0/1 touch (pre tiles
            # 0..7), with immediate norm_p2 so program order is write->read
            for w in ([("cur", 0)] + [("pre", i) for i in range(6)]
                      + [("cur", 1), ("pre", 6), ("pre", 7)]):
                do_chunk(w)
            # remaining stage-A chunks; a chunk's norm_p2 is emitted 2 blocks
            # after its p1, so the slack must be 3 tiles (9 rows) for the
            # 3-row pre tiles to be written before the gram that reads them
            due = {}
            hp, hc = 8, 2
            npairs_ = HSLICE // 2
            for t in range(npairs_):
                items = []
                need_cur = min((2 * t + 5) // CUR_RPT + 2, CUR_NT)
                need_pre = min((2 * t + 21) // PRE_RPT + 3, PRE_NT)
                while hc < need_cur:
                    items.append(("cur", hc)); hc += 1
                while hp < need_pre:
                    items.append(("pre", hp)); hp += 1
                due[t] = items

            _recq = []

            def emit_a(t):
                for w in due.get(t, []):
                    do_chunk_p1(w, t)
                    _recq.append(_p2q[-1][1])
                flush_p2(t)

            def emit_recips():
                while _recq:
                    norm_recips(_recq.pop(0))

            def prework(h):
                """gram + shear + aux DMA + y build; returns row state"""
                stage = stgp.tile([128, STG], bf16, tag="stage", name="stage")
                ct = cur_nb[0][h // CUR_RPT], cur_nb[1][h // CUR_RPT]
                co_ = slice((h % CUR_RPT) * W, (h % CUR_RPT + 1) * W)
                # batch gram rows sharing a pre tile into one matmul
                r = h
                while r <= h + KW - 1:
                    pt_ = r // PRE_RPT
                    r_end = min(pt_ * PRE_RPT + PRE_RPT - 1, h + KW - 1)
                    n = r_end - r + 1
                    po = (r % PRE_RPT) * WP
                    g = gp.tile([128, PRE_RPT * WP], f32, tag="g", name="g")
                    for c in range(2):
                        nc.tensor.matmul(
                            g[:, 0:n * WP], ct[c][:, co_],
                            pre_nb[c][pt_][:, po:po + n * WP],
                            start=(c == 0), stop=(c == 1))
                    dy0 = r - h
                    nc.scalar.activation(
                        stage[:, dy0 * WP:(dy0 + n) * WP],
                        g[:, 0:n * WP], AF.Copy)
                    r = r_end + 1
                sc = scr[h % 6]
                wview = sc[0:128 * (STG + 1)].rearrange("(p r) -> p r", r=STG + 1)
                nc.sync.dma_start(wview[:, 0:STG], stage[:, :])
                co = cop.tile([128, M4W], bf16, tag="co", name="co", bufs=3)
                rview = sc[:].rearrange("(p r) -> p r", r=STG + 2)
                rview = rview[:, 0:STG].rearrange("p (a b) -> p a b", b=WP)
                nc.sync.dma_start(co[:, 0:D], rview[:, :, 0:KW])
                axc = cop.tile([128, AUXT], i16, tag="axc", name="axc", bufs=8)
                nc.sync.dma_start(axc[:, :], auxc_d[h, :, :])
                y = yp.tile([128, M4W], f32, tag="y", name="y")
                nc.gpsimd.tensor_tensor(
                    out=y[:, 0:D], in0=co[:, 0:D],
                    in1=axc[:, AX_M4:AX_HM6].bitcast(bf16)[:, 0:D],
                    op=ALU.add)
                nc.gpsimd.memset(y[:, D:M4W], -9.0)
                return [y, None, axc]

            def phase_rounds(arr, lw, cs, r0, nr):
                for r in range(r0, r0 + nr):
                    nc.vector.max(cs[:, r * 8:(r + 1) * 8], arr[:, 0:lw])
                    if r != NROUND - 1:
                        nc.vector.match_replace(arr[:, 0:lw],
                                                cs[:, r * 8:(r + 1) * 8],
                                                arr[:, 0:lw], imm_value=-9.0)

            def compact_scan(arr, ci):
                """sign (Act) + counting scan (DVE): returns (sg unused, e-pre)"""
                w_in = CIN[ci]
                sg = cwp.tile([128, M4W], f32, tag="sg", name="sg")
                nc.scalar.activation(sg[:, 0:w_in], arr[:, 0:w_in],
                                     AF.Sign, bias=bias85[:, :])
                s = cwp.tile([128, M4W], f32, tag="s", name="s")
                nc.vector.tensor_tensor_scan(out=s[:, 0:w_in],
                                             data0=sg[:, 0:w_in],
                                             data1=ones290[:, 0:w_in],
                                             initial=0.0,
                                             op0=ALU.add, op1=ALU.add)
                return sg, s

            def compact_fin(arr, ci, sg, s):
                """index arithmetic (Pool), strided i16 casts (Act), u16-pair
                scatter (Pool).  Returns the compacted zc tile."""
                w_in = CIN[ci]
                e = cwp.tile([128, M4W], f32, tag="e", name="e")
                nc.gpsimd.tensor_scalar(out=e[:, 0:w_in], in0=sg[:, 0:w_in],
                                        scalar1=10000.0, scalar2=None,
                                        op0=ALU.mult)
                nc.gpsimd.tensor_tensor(out=e[:, 0:w_in], in0=e[:, 0:w_in],
                                        in1=s[:, 0:w_in], op=ALU.add)
                i2 = i2p.tile([128, 2 * M4W], i16, tag="i2", name="i2")
                i2v = i2[:, 0:2 * w_in].rearrange("p (n two) -> p n two",
                                                  two=2)
                nc.scalar.activation(i2v[:, :, 0], e[:, 0:w_in], AF.Copy,
                                     bias=-10002.0)
                nc.scalar.activation(i2v[:, :, 1], e[:, 0:w_in], AF.Copy,
                                     bias=-10001.0)
                zc = zcpools[ci].tile([128, ZCC[ci]], f32, tag=f"zc{ci}",
                                      name=f"zc{ci}")
                nc.gpsimd.local_scatter(
                    zc[:, :].bitcast(u16), arr[:, 0:w_in].bitcast(u16),
                    i2[:, 0:2 * w_in], channels=128,
                    num_elems=2 * ZCC[ci], num_idxs=2 * w_in)
                return zc

            def pmax(out, a, bb, d, r):
                """out = max(a, b) without DVE: d = b - a (Pool),
                r = relu(d) (Act), out = a + r (Pool)."""
                nc.gpsimd.tensor_tensor(out=d[:, :], in0=bb[:, :], in1=a[:, :],
                                        op=ALU.subtract)
                nc.scalar.activation(r[:, :], d[:, :], AF.Relu)
                nc.gpsimd.tensor_tensor(out=out[:, :], in0=a[:, :],
                                        in1=r[:, :], op=ALU.add)

            def postwork(h, state):
                """casts, scatters, combines, transpose, combined output DMA"""
                _, cs, axc = state
                cs4b = rcp.tile([128, TOPK], bf16, tag="cs4b", name="cs4b")
                nc.scalar.activation(cs4b[:, :], cs[:, 0:TOPK], AF.Copy,
                                     bias=-4.0)
                csb = rcp.tile([128, TOPK], bf16, tag="csb", name="csb")
                nc.scalar.activation(csb[:, :], cs[:, 0:TOPK], AF.Copy)
                tf = rcp.tile([128, TOPK], bf16, tag="tf", name="tf")
                nc.gpsimd.local_scatter(tf[:, :], cs4b[:, :],
                                        axc[:, AX_IDXF:AX_IDXB], channels=128,
                                        num_elems=TOPK, num_idxs=TOPK)
                yb = rcp.tile([128, TOPK], bf16, tag="yb", name="yb")
                nc.gpsimd.local_scatter(yb[:, :], csb[:, :],
                                        axc[:, AX_IDXB:AUXT], channels=128,
                                        num_elems=TOPK, num_idxs=TOPK)
                axb = axc[:, 0:AX_IDXF].bitcast(bf16)
                # f = max(cs4b - hm6, min(tf, 0))
                fmain = fbp.tile([128, TOPK], bf16, tag="fm", name="fm")
                nc.gpsimd.tensor_tensor(out=fmain[:, :], in0=cs4b[:, :],
                                        in1=axb[:, AX_HM6:AX_HM10],
                                        op=ALU.subtract)
                nc.gpsimd.tensor_scalar_min(tf[:, :], tf[:, :], 0.0)
                ft = fbp.tile([128, TOPK], bf16, tag="ft", name="ft")
                dd = fbp.tile([128, TOPK], f32, tag="dd", name="dd")
                rr = fbp.tile([128, TOPK], f32, tag="rr", name="rr")
                pmax(ft, fmain, tf, dd, rr)
                # b = max(yb - hm10, min(csb, 0))
                bmain = fbp.tile([128, TOPK], bf16, tag="bm", name="bm")
                nc.gpsimd.tensor_tensor(out=bmain[:, :], in0=yb[:, :],
                                        in1=axb[:, AX_HM10:AX_IDXF],
                                        op=ALU.subtract)
                minb = fbp.tile([128, TOPK], bf16, tag="mb", name="mb")
                nc.gpsimd.tensor_scalar_min(minb[:, :], csb[:, :], 0.0)
                bt = fbp.tile([128, TOPK], bf16, tag="bt", name="bt")
                dd2 = fbp.tile([128, TOPK], f32, tag="dd2", name="dd2")
                rr2 = fbp.tile([128, TOPK], f32, tag="rr2", name="rr2")
                pmax(bt, bmain, minb, dd2, rr2)
                tr = trp.tile([128, 2 * TOPK], f32, tag="tr", name="tr")
                for oi, x in ((0, ft), (1, bt)):
                    for half in range(2):
                        pt = tp.tile([128, 128], bf16, tag="pt", name="pt")
                        nc.tensor.transpose(
                            pt[:, :], x[:, half * 128:(half + 1) * 128],
                            ident[:, :])
                        base = oi * TOPK + half * 128
                        nc.scalar.activation(tr[:, base:base + 128], pt[:, :],
                                             AF.Copy)
                oview = fb_d.rearrange("two (cc p) hh ww -> p two cc hh ww",
                                       cc=2)
                nc.sync.dma_start(
                    oview[:, :, :, h, :],
                    tr[:, :].rearrange("p (two cc ww) -> p two cc ww",
                                       two=2, ww=W))

            npairs = HSLICE // 2

            # two-pair software pipeline: pair t runs phases P0+P1 in block t
            # and P2+P3 in block t+1, so every compaction chain (Act sign ->
            # DVE scan -> Pool index math -> Act casts -> Pool scatter) hides
            # behind ~8-9us of the other pair's DVE rounds.  prework for pair
            # t+2 is emitted mid-block so its Act stage copies queue after the
            # signs the DVE scans are waiting on.
            pend = {0: (prework(0), prework(1)), 1: (prework(2), prework(3))}
            mid = {}
            for t in range(npairs + 1):
                cur = pend.pop(t, None)
                old = mid.pop(t - 1, None)
                # cur P0 (rows A then B), then the two c1 scans
                if cur is not None:
                    for st in cur:
                        st[1] = csp.tile([128, CS_COLS], f32, tag="cs",
                                         name="cs")
                        phase_rounds(st[0], LIVE[0], st[1], 0, PHR[0])
                    scans0 = [compact_scan(st[0], 0) for st in cur]
                # old P2 + c3 scans
                if old is not None:
                    for ri, st in enumerate(old["st"]):
                        phase_rounds(old["arrs"][ri], LIVE[2], st[1], 16,
                                     PHR[2])
                    scans2 = [compact_scan(old["arrs"][ri], 2)
                              for ri in range(2)]
                # cur c1 fins -> zc1
                if cur is not None:
                    arrs1 = [compact_fin(cur[ri][0], 0, *scans0[ri])
                             for ri in range(2)]
                # norm chunks + prework for pair t+2: AFTER the signs/casts so
                # the Act queue serves the sort's compaction chain first (the
                # stage copies have ~3 blocks of slack)
                if t + 2 <= npairs - 1:
                    emit_a(t)
                    pend[t + 2] = (prework(2 * t + 4), prework(2 * t + 5))
                emit_recips()
                # cur P1 + c2 scans
                if cur is not None:
                    for ri, st in enumerate(cur):
                        phase_rounds(arrs1[ri], LIVE[1], st[1], 8, PHR[1])
                    scans1 = [compact_scan(arrs1[ri], 1) for ri in range(2)]
                # old c3 fins -> zc3, then P3
                if old is not None:
                    arrs3 = [compact_fin(old["arrs"][ri], 2, *scans2[ri])
                             for ri in range(2)]
                    for ri, st in enumerate(old["st"]):
                        phase_rounds(arrs3[ri], LIVE[3], st[1], 24, PHR[3])
                # cur c2 fins -> zc2 (consumed next block)
                if cur is not None:
                    arrs2 = [compact_fin(arrs1[ri], 1, *scans1[ri])
                             for ri in range(2)]
                    mid[t] = {"st": cur, "arrs": arrs2}
                # old postwork
                if old is not None:
                    postwork(2 * (t - 1), old["st"][0])
                    postwork(2 * (t - 1) + 1, old["st"][1])
            for t in range(npairs - 2, npairs_):
                emit_a(t)
            flush_p2(10**6)
    nc.compile()
    return nc


def _host_prep(pre, curr, mask):
    import ml_dtypes
    bf = ml_dtypes.bfloat16

    pre_pad = np.pad(pre, ((0, 0), (0, 0), (K, K), (K, K)), mode="reflect")
    mask_pad = np.pad(mask, ((0, 0), (0, 0), (K, K), (K, K)))
    ins = []
    jf = np.arange(TOPK, dtype=np.int32)[None, :]
    for k in range(N_CORES):
        b, hh = k // 2, k % 2
        h0 = hh * HSLICE
        mp = mask_pad[b, 0, h0:h0 + HP, :]
        s0, s1 = mp.strides
        m_unf = np.lib.stride_tricks.as_strided(
            mp, (HSLICE, KW, W, KW), (s0, s0, s1, s1))
        m_unf = np.ascontiguousarray(
            m_unf.transpose(0, 2, 1, 3).reshape(HSLICE, W, D))
        kap = m_unf.sum(axis=2).astype(np.int32).reshape(-1, 1)   # [HS*W, 1]
        q = D - kap
        idxf = np.where((jf < kap) & (jf + q < TOPK), jf + q, -1)
        idxb = np.where(jf >= kap, jf - kap, -1)
        auxi = np.concatenate([idxf, idxb], axis=1).astype(np.int16)
        hm6 = np.where(jf >= kap, np.float32(6.0), np.float32(0.0))
        hm10 = np.where((jf >= q) | (jf >= TOPK - kap),
                        np.float32(10.0), np.float32(0.0))
        m4 = np.full((HSLICE * W, M4W), -9.0, np.float32)
        m4[:, :D] = 4.0 * m_unf.reshape(-1, D)
        # y = co + m4: pad col 289 gets co_pad(=0 in bf16 tile) + (-9) = -9...
        # co tile col 289 is uninitialized; the kernel memsets y[:,289] anyway.
        m4[:, D] = 0.0
        auxb = np.concatenate([m4, hm6, hm10], axis=1).astype(bf)
        auxc = np.concatenate([auxb.view(np.int16), auxi], axis=1)
        ins.append({
            "pre_pad": np.ascontiguousarray(pre_pad[b, :, h0:h0 + HP, :]),
            "curr": np.ascontiguousarray(curr[b, :, h0:h0 + HSLICE, :]),
            "auxc": np.ascontiguousarray(auxc.reshape(HSLICE, W, AUXT)),
        })
    return ins


def kernel(pre, curr, mask, mode):
    from concourse.bass_utils import run_bass_kernel_spmd

    pre = np.asarray(pre, dtype=np.float32)
    curr = np.asarray(curr, dtype=np.float32)
    mask = np.asarray(mask, dtype=np.float32)
    assert int(np.asarray(mode)) == 0

    if _CACHED["nc"] is None:
        _CACHED["nc"] = _build_nc()
    nc = _CACHED["nc"]

    in_maps = _host_prep(pre, curr, mask)
    res = run_bass_kernel_spmd(nc, in_maps, core_ids=list(range(N_CORES)))
    f = np.zeros((B, TOPK, H, W), np.float32)
    bo = np.zeros((B, TOPK, H, W), np.float32)
    for k in range(N_CORES):
        bb, hh = k // 2, k % 2
        fb = res.results[k]["fb_out"]
        f[bb, :, hh * HSLICE:(hh + 1) * HSLICE, :] = fb[0]
        bo[bb, :, hh * HSLICE:(hh + 1) * HSLICE, :] = fb[1]
    return (f, bo)
